# revision 5
# baseline (speedup 1.0000x reference)
"""Cross-attention (single-head, residual) Bass/Tile kernel for Trainium2.

Problem: y = x + (softmax((x' Wq + bq)(ctx Wk + bk)^T / sqrt(C)) (ctx Wv + bv)) Wo + bo
  x: [B=8, C=512, H=64, W=64], context: [B=8, Lc=512, CTX=768]

Sharding: pure data-parallel over batch - one batch element per NeuronCore,
no collectives.

Algebraic restructuring (saves ~1/3 of the matmul work): with
  kT = (ctx Wk + bk)^T           [C, Lc]
  G  = Wq kT                     [C', Lc]  (Wq folded into the key side)
  vW = (ctx Wv + bv) Wo + 1 bo^T [Lc, C]   (Wo and bo folded into the V side;
                                            exact because softmax rows sum to 1)
the streaming loop per hw-tile is two matmul stages plus a softmax row-sum:
  simT = G^T x                   [Lc, hw]
  eT   = exp(scale*simT + scale*kT^T bq)   (bq folded into the ACT bias)
  y    = (vW^T eT) * (1/colsum(eT)) + x

All layout work (transposes, fp8/bf16 casts, channel-chunk packing) happens
on the host; the device only runs matmuls, ACT exp/identity evictions, the
reciprocal, and the residual multiply-adds.  Weights are shipped as
fp8e4m3 scaled by 32 (raw std 0.02 is fp8-subnormal territory); the 32s
cancel against a constant-32 stationary matrix in the softmax denominator
matmul, which simultaneously computes the column sum and broadcasts it
across all 128 partitions in a single DoubleRow matmul.
"""

import numpy as np

B = 8
C = 512
CTX = 768
Lc = 512
HH = 64
WW = 64
HW = HH * WW          # 4096
N_CORES = 8
P = 128
HT = 512              # hw tile (free-dim) width
N_HT = HW // HT       # 8
KC = C // P           # 4
KX = CTX // P         # 6
KL = Lc // P          # 4
SCALE = float(C) ** -0.5
WS = 32.0             # fp8 weight pre-scale

_cache = {}


def _build_nc():
    import concourse.mybir as mybir
    import concourse.bass as bass
    import concourse.tile as tile
    from concourse import bacc

    f32 = mybir.dt.float32
    bf16 = mybir.dt.bfloat16
    fp8 = mybir.dt.float8e4
    AF = mybir.ActivationFunctionType
    DR = mybir.MatmulPerfMode.DoubleRow

    nc = bacc.Bacc("TRN2", target_bir_lowering=False, debug=False,
                   num_devices=N_CORES)

    # host-prepped layouts: [128(part), chunk, free]
    x8_d = nc.dram_tensor("x8", [P, N_HT, KC, HT], fp8, kind="ExternalInput").ap()
    xb_d = nc.dram_tensor("xb", [P, N_HT, KC, HT], bf16, kind="ExternalInput").ap()
    ctxT_d = nc.dram_tensor("ctxt8", [P, KX, Lc], fp8, kind="ExternalInput").ap()
    wk_d = nc.dram_tensor("wk8", [P, KX, C], fp8, kind="ExternalInput").ap()
    wv_d = nc.dram_tensor("wv8", [P, KX, C], fp8, kind="ExternalInput").ap()
    wqT_d = nc.dram_tensor("wqt8", [P, KC, C], fp8, kind="ExternalInput").ap()
    wo_d = nc.dram_tensor("wo8", [P, KC, C], fp8, kind="ExternalInput").ap()
    bq_d = nc.dram_tensor("bq8", [P, KC], fp8, kind="ExternalInput").ap()
    bkv_d = nc.dram_tensor("bkv", [P, 2 * KC], f32, kind="ExternalInput").ap()
    bo_d = nc.dram_tensor("bo32", [1, C], f32, kind="ExternalInput").ap()
    y_d = nc.dram_tensor("y", [P, N_HT, KC, HT], bf16, kind="ExternalOutput").ap()

    with tile.TileContext(nc) as tc:
        with (
            tc.tile_pool(name="const", bufs=1) as const,
            tc.tile_pool(name="x8in", bufs=N_HT) as x8in,
            tc.tile_pool(name="xbin", bufs=N_HT) as xbin,
            tc.tile_pool(name="work", bufs=2) as work,
            tc.tile_pool(name="yout", bufs=2) as yout,
            tc.tile_pool(name="psum_s", bufs=3, space="PSUM") as psum_s,
            tc.tile_pool(name="psum_bc", bufs=2, space="PSUM") as psum_bc,
            tc.tile_pool(name="psum_y", bufs=3, space="PSUM") as psum_y,
        ):
            # ------------- DMAs -------------
            # Each dma_start lands on ONE hw queue at ~77ns/descriptor
            # (1 descriptor per partition), so big loads are partition-split
            # across several dma_starts to run queues in parallel.  The
            # critical kT operands (ctxT8+wk8) go first, 4-way split.
            ctxT8 = const.tile([P, KX, Lc], fp8, name="ctxT8", tag="ctxT8")
            wk8 = const.tile([P, KX, C], fp8, name="wk8", tag="wk8")
            for q in range(4):
                s = slice(32 * q, 32 * (q + 1))
                nc.sync.dma_start(out=ctxT8[s], in_=ctxT_d[s])
                nc.sync.dma_start(out=wk8[s], in_=wk_d[s])
            wv8 = const.tile([P, KX, C], fp8, name="wv8", tag="wv8")
            wqT8 = const.tile([P, KC, C], fp8, name="wqT8", tag="wqT8")
            wo8 = const.tile([P, KC, C], fp8, name="wo8", tag="wo8")
            for q in range(2):
                s = slice(64 * q, 64 * (q + 1))
                nc.sync.dma_start(out=wv8[s], in_=wv_d[s])
                nc.sync.dma_start(out=wqT8[s], in_=wqT_d[s])
                nc.sync.dma_start(out=wo8[s], in_=wo_d[s])

            # x tiles: all 16 loads issued upfront on the Activation HWDGE
            # (its own 16 hw queues, otherwise idle) so the streaming loop
            # never waits on input DMA.
            x8_tiles, xb_tiles = {}, {}
            for h in range(N_HT):
                x8 = x8in.tile([P, KC, HT], fp8, tag="x8", name=f"x8_{h}")
                nc.scalar.dma_start(out=x8, in_=x8_d[:, h])
                x8_tiles[h] = x8
                xb = xbin.tile([P, KC, HT], bf16, tag="xb", name=f"xb_{h}")
                nc.scalar.dma_start(out=xb, in_=xb_d[:, h])
                xb_tiles[h] = xb

            # biases on the gpsimd DGE queue (tiny; keeps SP queues clean)
            bq8 = const.tile([P, KC], fp8, name="bq8", tag="bq8")
            bkv = const.tile([P, 2 * KC], f32, name="bkv", tag="bkv")
            nc.gpsimd.dma_start(out=bq8, in_=bq_d)
            nc.gpsimd.dma_start(out=bkv, in_=bkv_d)
            # 32*bo broadcast across partitions via stride-0 DMA
            bo_bc = const.tile([P, C], f32, name="bo_bc", tag="bo_bc")
            bo_src = bass.AP(tensor=bo_d.tensor, offset=bo_d.offset,
                             ap=[[0, P]] + list(bo_d.ap)[1:])
            nc.gpsimd.dma_start(out=bo_bc, in_=bo_src)

            # constant-32 stationary matrix for the fused colsum+broadcast
            ones32 = const.tile([P, 2, P], fp8, name="ones32", tag="ones32")
            nc.vector.memset(ones32, WS)

            # prewarm the ACT exp table so the one-time table load hides
            # under the initial weight DMAs
            warm = const.tile([1, 1], f32, name="warm", tag="warm")
            nc.scalar.activation(warm, ones32[0:1, 0, 0:1], AF.Exp)

            # ------------- phase A -------------
            # kT [128(c), KC, Lc] = (ctx Wk + bk)^T   (fp8, unscaled)
            kT8 = const.tile([P, KC, Lc], fp8, name="kT8", tag="kT8")
            for mc in range(KC):
                ps = psum_s.tile([P, Lc], f32, tag="ps_s", name=f"ps_k_{mc}")
                for u in range(KX // 2):
                    nc.tensor.matmul(ps,
                                     wk8[:, 2 * u:2 * u + 2, mc * P:(mc + 1) * P],
                                     ctxT8[:, 2 * u:2 * u + 2, :],
                                     start=(u == 0), stop=(u == KX // 2 - 1),
                                     perf_mode=DR)
                nc.scalar.activation(kT8[:, mc, :], ps, AF.Identity,
                                     scale=1.0 / WS, bias=bkv[:, mc:mc + 1])

            # vT [128(c), KC, Lc] = (ctx Wv + bv)^T   (fp8, unscaled)
            vT8 = const.tile([P, KC, Lc], fp8, name="vT8", tag="vT8")
            for mc in range(KC):
                ps = psum_y.tile([P, Lc], f32, tag="ps_y", name=f"ps_v_{mc}")
                for u in range(KX // 2):
                    nc.tensor.matmul(ps,
                                     wv8[:, 2 * u:2 * u + 2, mc * P:(mc + 1) * P],
                                     ctxT8[:, 2 * u:2 * u + 2, :],
                                     start=(u == 0), stop=(u == KX // 2 - 1),
                                     perf_mode=DR)
                nc.scalar.activation(vT8[:, mc, :], ps, AF.Identity,
                                     scale=1.0 / WS, bias=bkv[:, KC + mc:KC + mc + 1])

            # G [128(c'), KC, Lc] = 32 * Wq kT   (fp8; the 32 comes from wqT8
            # and is folded into the exp scale)
            G8 = const.tile([P, KC, Lc], fp8, name="G8", tag="G8")
            for mg in range(KC):
                ps = psum_s.tile([P, Lc], f32, tag="ps_s", name=f"ps_g_{mg}")
                for u in range(KC // 2):
                    nc.tensor.matmul(ps,
                                     wqT8[:, 2 * u:2 * u + 2, mg * P:(mg + 1) * P],
                                     kT8[:, 2 * u:2 * u + 2, :],
                                     start=(u == 0), stop=(u == KC // 2 - 1),
                                     perf_mode=DR)
                nc.vector.tensor_copy(out=G8[:, mg, :], in_=ps)

            # bqk_s [128(lc), KL] = SCALE * kT^T bq   (per-lc exp bias)
            bqk_s = const.tile([P, KL], f32, name="bqk_s", tag="bqk")
            for ml in range(KL):
                ps = psum_bc.tile([P, HT], f32, tag="ps_bc", name=f"ps_bq_{ml}")
                for mc in range(KC):
                    nc.tensor.matmul(ps[:, 0:1],
                                     kT8[:, mc, ml * P:(ml + 1) * P],
                                     bq8[:, mc:mc + 1],
                                     start=(mc == 0), stop=(mc == KC - 1))
                nc.scalar.activation(bqk_s[:, ml:ml + 1], ps[:, 0:1],
                                     AF.Identity, scale=SCALE)

            # vW [128(lc), KL, C] = 32*((v+bv) Wo + 1 bo^T)  (fp8; 32 from wo8,
            # cancelled by the 32 in the denominator matmul)
            vW8 = const.tile([P, KL, C], fp8, name="vW8", tag="vW8")
            for ml in range(KL):
                ps = psum_y.tile([P, C], f32, tag="ps_y", name=f"ps_vw_{ml}")
                for u in range(KC // 2):
                    nc.tensor.matmul(ps,
                                     vT8[:, 2 * u:2 * u + 2, ml * P:(ml + 1) * P],
                                     wo8[:, 2 * u:2 * u + 2, :],
                                     start=(u == 0), stop=(u == KC // 2 - 1),
                                     perf_mode=DR)
                nc.vector.tensor_add(out=vW8[:, ml, :], in0=ps, in1=bo_bc)

            # ------------- phase B: stream over hw tiles -------------
            # software pipeline: y-matmuls of tile h-1 are emitted after
            # simT of tile h, so the PE never waits on the exp evictions
            prev = None

            def emit_y(h, eT, rec_sb, xb):
                y_sb = yout.tile([P, KC, HT], bf16, tag="y", name=f"y_{h}")
                for mo in range(KC):
                    ps = psum_y.tile([P, HT], f32, tag="ps_y", name=f"ps_y_{h}_{mo}")
                    for u in range(KL // 2):
                        nc.tensor.matmul(ps,
                                         vW8[:, 2 * u:2 * u + 2, mo * P:(mo + 1) * P],
                                         eT[:, 2 * u:2 * u + 2, :],
                                         start=(u == 0), stop=(u == KL // 2 - 1),
                                         perf_mode=DR)
                    nc.vector.tensor_mul(out=y_sb[:, mo, :], in0=ps, in1=rec_sb)
                    add_eng = nc.vector if mo >= 2 else nc.gpsimd
                    add_eng.tensor_add(out=y_sb[:, mo, :], in0=y_sb[:, mo, :],
                                       in1=xb[:, mo, :])
                    for q in range(4):
                        s = slice(32 * q, 32 * (q + 1))
                        nc.sync.dma_start(out=y_d[s, h, mo], in_=y_sb[s, mo, :])

            for h in range(N_HT):
                x8 = x8_tiles[h]

                # eT [lc, hw] = exp(scale*(G^T x) + scale*kT^T bq)
                eT = work.tile([P, KL, HT], fp8, tag="eT", name=f"eT_{h}")
                for ml in range(KL):
                    ps = psum_s.tile([P, HT], f32, tag="ps_s", name=f"ps_s_{h}_{ml}")
                    for u in range(KC // 2):
                        nc.tensor.matmul(ps,
                                         G8[:, 2 * u:2 * u + 2, ml * P:(ml + 1) * P],
                                         x8[:, 2 * u:2 * u + 2, :],
                                         start=(u == 0), stop=(u == KC // 2 - 1),
                                         perf_mode=DR)
                    nc.scalar.activation(eT[:, ml, :], ps, AF.Exp, scale=SCALE / WS,
                                         bias=bqk_s[:, ml:ml + 1])

                if prev is not None:
                    emit_y(*prev)

                # fused softmax denominator: one DR matmul pass over eT both
                # sums over lc and broadcasts 32*colsum to all 128 partitions
                ps_bc = psum_bc.tile([P, HT], f32, tag="ps_bc", name=f"ps_bc_{h}")
                for u in range(KL // 2):
                    nc.tensor.matmul(ps_bc, ones32, eT[:, 2 * u:2 * u + 2, :],
                                     start=(u == 0), stop=(u == KL // 2 - 1),
                                     perf_mode=DR)
                rec_sb = work.tile([P, HT], f32, tag="rec", name=f"rec_{h}")
                nc.vector.reciprocal_approx_fast(out=rec_sb, in_=ps_bc)
                prev = (h, eT, rec_sb, xb_tiles[h])
            emit_y(*prev)

    nc.compile()
    return nc


def _get_compiled():
    if "nc" not in _cache:
        _cache["nc"] = _build_nc()
    return _cache["nc"]


def _make_in_maps(x, context, Wq, bq, Wk, bk, Wv, bv, Wo, bo):
    import ml_dtypes
    fp8 = ml_dtypes.float8_e4m3
    bf16 = ml_dtypes.bfloat16

    x = np.asarray(x, dtype=np.float32)
    context = np.asarray(context, dtype=np.float32)
    Wq = np.asarray(Wq, dtype=np.float32)
    Wk = np.asarray(Wk, dtype=np.float32)
    Wv = np.asarray(Wv, dtype=np.float32)
    Wo = np.asarray(Wo, dtype=np.float32)

    def chunked(a, k):     # [k*128, F] -> [128, k, F]
        return np.ascontiguousarray(
            a.reshape(k, P, -1).transpose(1, 0, 2))

    common = {
        "wk8": chunked(WS * Wk, KX).astype(fp8),
        "wv8": chunked(WS * Wv, KX).astype(fp8),
        "wqt8": chunked(WS * Wq.T, KC).astype(fp8),
        "wo8": chunked(WS * Wo, KC).astype(fp8),
        "bq8": np.ascontiguousarray(
            np.asarray(bq, np.float32).reshape(KC, P).T).astype(fp8),
        "bkv": np.ascontiguousarray(np.stack(
            [np.asarray(bk, np.float32).reshape(KC, P),
             np.asarray(bv, np.float32).reshape(KC, P)],
        ).reshape(2 * KC, P).T),
        "bo32": np.ascontiguousarray(
            WS * np.asarray(bo, np.float32).reshape(1, C)),
    }
    in_maps = []
    for b in range(B):
        m = dict(common)
        # x2[c, hw] with c = ko*128+p  ->  [p, h, ko, j]
        x2 = x[b].reshape(KC, P, N_HT, HT).transpose(1, 2, 0, 3)
        m["x8"] = np.ascontiguousarray(x2).astype(fp8)
        m["xb"] = np.ascontiguousarray(x2).astype(bf16)
        # ctxT[cx, lc] with cx = cxo*128+p  ->  [p, cxo, lc]
        m["ctxt8"] = chunked(context[b].T, KX).astype(fp8)
        in_maps.append(m)
    return in_maps


def _run(in_maps, trace=False):
    from concourse.bass_utils import run_bass_kernel_spmd
    nc = _get_compiled()
    return run_bass_kernel_spmd(nc, in_maps, core_ids=list(range(N_CORES)),
                                trace=trace)


def _assemble(res):
    out = np.empty((B, C, HH, WW), np.float32)
    for b in range(B):
        yb = np.asarray(res.results[b]["y"], dtype=np.float32)
        # [p, h, ko, j] -> [ko, p, h, j] -> [C, HW]
        out[b] = yb.transpose(2, 0, 1, 3).reshape(C, HH, WW)
    return out


def kernel(x, context, Wq, bq, Wk, bk, Wv, bv, Wo, bo):
    in_maps = _make_in_maps(x, context, Wq, bq, Wk, bk, Wv, bv, Wo, bo)
    res = _run(in_maps, trace=False)
    return _assemble(res)


# revision 7
# speedup vs baseline: 1.5516x; 1.5516x over previous
"""Cross-attention (single-head, residual) Bass/Tile kernel for Trainium2.

Problem: y = x + (softmax((x' Wq + bq)(ctx Wk + bk)^T / sqrt(C)) (ctx Wv + bv)) Wo + bo
  x: [B=8, C=512, H=64, W=64], context: [B=8, Lc=512, CTX=768]

Sharding: pure data-parallel over batch - one batch element per NeuronCore,
no collectives.

Algebraic restructuring (saves ~1/3 of the matmul work): with
  kT = (ctx Wk + bk)^T           [C, Lc]
  G  = Wq kT                     [C', Lc]  (Wq folded into the key side)
  vW = (ctx Wv + bv) Wo + 1 bo^T [Lc, C]   (Wo and bo folded into the V side;
                                            exact because softmax rows sum to 1)
the streaming loop per hw-tile is two matmul stages plus a softmax row-sum:
  simT = G^T x                   [Lc, hw]
  eT   = exp(scale*simT + scale*kT^T bq)   (bq folded into the ACT bias)
  y    = (vW^T eT) * (1/colsum(eT)) + x

All layout work (transposes, fp8/bf16 casts, channel-chunk packing) happens
on the host; the device only runs matmuls, ACT exp/identity evictions, the
reciprocal, and the residual multiply-adds.  Weights are shipped as
fp8e4m3 scaled by 32 (raw std 0.02 is fp8-subnormal territory); the 32s
cancel against a constant-32 stationary matrix in the softmax denominator
matmul, which simultaneously computes the column sum and broadcasts it
across all 128 partitions in a single DoubleRow matmul.
"""

import numpy as np

B = 8
C = 512
CTX = 768
Lc = 512
HH = 64
WW = 64
HW = HH * WW          # 4096
N_CORES = 8
P = 128
HT = 512              # hw tile (free-dim) width
N_HT = HW // HT       # 8
KC = C // P           # 4
KX = CTX // P         # 6
KL = Lc // P          # 4
SCALE = float(C) ** -0.5
WS = 32.0             # fp8 weight pre-scale

_cache = {}


def _build_nc():
    import concourse.mybir as mybir
    import concourse.bass as bass
    import concourse.tile as tile
    from concourse import bacc

    f32 = mybir.dt.float32
    bf16 = mybir.dt.bfloat16
    fp8 = mybir.dt.float8e4
    AF = mybir.ActivationFunctionType
    DR = mybir.MatmulPerfMode.DoubleRow

    nc = bacc.Bacc("TRN2", target_bir_lowering=False, debug=False,
                   num_devices=N_CORES)

    # host-prepped layouts: [128(part), chunk, free]
    x8_d = nc.dram_tensor("x8", [P, N_HT, KC, HT], fp8, kind="ExternalInput").ap()
    xb_d = nc.dram_tensor("xb", [P, N_HT, KC, HT], bf16, kind="ExternalInput").ap()
    ctxT_d = nc.dram_tensor("ctxt8", [P, KX, Lc], fp8, kind="ExternalInput").ap()
    wk_d = nc.dram_tensor("wk8", [P, KX, C], fp8, kind="ExternalInput").ap()
    wv_d = nc.dram_tensor("wv8", [P, KX, C], fp8, kind="ExternalInput").ap()
    wqT_d = nc.dram_tensor("wqt8", [P, KC, C], fp8, kind="ExternalInput").ap()
    wo_d = nc.dram_tensor("wo8", [P, KC, C], fp8, kind="ExternalInput").ap()
    bq_d = nc.dram_tensor("bq8", [P, KC], fp8, kind="ExternalInput").ap()
    bkv_d = nc.dram_tensor("bkv", [P, 2 * KC], f32, kind="ExternalInput").ap()
    bo_d = nc.dram_tensor("bo32", [1, C], f32, kind="ExternalInput").ap()
    y_d = nc.dram_tensor("y", [P, N_HT, KC, HT], bf16, kind="ExternalOutput").ap()

    with tile.TileContext(nc) as tc:
        with (
            tc.tile_pool(name="const", bufs=1) as const,
            tc.tile_pool(name="x8in", bufs=N_HT) as x8in,
            tc.tile_pool(name="xbin", bufs=N_HT) as xbin,
            tc.tile_pool(name="work", bufs=2) as work,
            tc.tile_pool(name="yout", bufs=2) as yout,
            tc.tile_pool(name="psum_s", bufs=3, space="PSUM") as psum_s,
            tc.tile_pool(name="psum_bc", bufs=2, space="PSUM") as psum_bc,
            tc.tile_pool(name="psum_y", bufs=3, space="PSUM") as psum_y,
        ):
            # ------------- DMAs -------------
            # Each dma_start lands on ONE hw queue at ~77ns/descriptor
            # (1 descriptor per partition), so big loads are partition-split
            # across several dma_starts to run queues in parallel.  The
            # critical kT operands (ctxT8+wk8) go first, 4-way split.
            ctxT8 = const.tile([P, KX, Lc], fp8, name="ctxT8", tag="ctxT8")
            wk8 = const.tile([P, KX, C], fp8, name="wk8", tag="wk8")
            for q in range(4):
                s = slice(32 * q, 32 * (q + 1))
                nc.sync.dma_start(out=ctxT8[s], in_=ctxT_d[s])
                nc.sync.dma_start(out=wk8[s], in_=wk_d[s])
            wv8 = const.tile([P, KX, C], fp8, name="wv8", tag="wv8")
            wqT8 = const.tile([P, KC, C], fp8, name="wqT8", tag="wqT8")
            wo8 = const.tile([P, KC, C], fp8, name="wo8", tag="wo8")
            for q in range(2):
                s = slice(64 * q, 64 * (q + 1))
                nc.sync.dma_start(out=wv8[s], in_=wv_d[s])
                nc.sync.dma_start(out=wqT8[s], in_=wqT_d[s])
                nc.sync.dma_start(out=wo8[s], in_=wo_d[s])

            # x tiles: all loads issued upfront (after the weights, same
            # engine, so queue FIFOs drain in dependency order), 2-way
            # partition-split to run two queues per load.
            x8_tiles, xb_tiles = {}, {}
            for h in range(N_HT):
                x8 = x8in.tile([P, KC, HT], fp8, tag="x8", name=f"x8_{h}")
                xb = xbin.tile([P, KC, HT], bf16, tag="xb", name=f"xb_{h}")
                for q in range(2):
                    s = slice(64 * q, 64 * (q + 1))
                    nc.sync.dma_start(out=x8[s], in_=x8_d[s, h])
                    nc.sync.dma_start(out=xb[s], in_=xb_d[s, h])
                x8_tiles[h] = x8
                xb_tiles[h] = xb

            # biases on the gpsimd DGE queue (tiny; keeps SP queues clean)
            bq8 = const.tile([P, KC], fp8, name="bq8", tag="bq8")
            bkv = const.tile([P, 2 * KC], f32, name="bkv", tag="bkv")
            nc.gpsimd.dma_start(out=bq8, in_=bq_d)
            nc.gpsimd.dma_start(out=bkv, in_=bkv_d)
            # 32*bo broadcast across partitions via stride-0 DMA
            bo_bc = const.tile([P, C], f32, name="bo_bc", tag="bo_bc")
            bo_src = bass.AP(tensor=bo_d.tensor, offset=bo_d.offset,
                             ap=[[0, P]] + list(bo_d.ap)[1:])
            nc.gpsimd.dma_start(out=bo_bc, in_=bo_src)

            # constant-32 stationary matrix for the fused colsum+broadcast
            ones32 = const.tile([P, 2, P], fp8, name="ones32", tag="ones32")
            nc.vector.memset(ones32, WS)

            # prewarm the ACT exp table so the one-time table load hides
            # under the initial weight DMAs
            warm = const.tile([1, 1], f32, name="warm", tag="warm")
            nc.scalar.activation(warm, ones32[0:1, 0, 0:1], AF.Exp)

            # ------------- phase A -------------
            # kT [128(c), KC, Lc] = (ctx Wk + bk)^T   (fp8, unscaled)
            kT8 = const.tile([P, KC, Lc], fp8, name="kT8", tag="kT8")
            for mc in range(KC):
                ps = psum_s.tile([P, Lc], f32, tag="ps_s", name=f"ps_k_{mc}")
                for u in range(KX // 2):
                    nc.tensor.matmul(ps,
                                     wk8[:, 2 * u:2 * u + 2, mc * P:(mc + 1) * P],
                                     ctxT8[:, 2 * u:2 * u + 2, :],
                                     start=(u == 0), stop=(u == KX // 2 - 1),
                                     perf_mode=DR)
                nc.scalar.activation(kT8[:, mc, :], ps, AF.Identity,
                                     scale=1.0 / WS, bias=bkv[:, mc:mc + 1])

            # vT [128(c), KC, Lc] = (ctx Wv + bv)^T   (fp8, unscaled)
            vT8 = const.tile([P, KC, Lc], fp8, name="vT8", tag="vT8")
            for mc in range(KC):
                ps = psum_y.tile([P, Lc], f32, tag="ps_y", name=f"ps_v_{mc}")
                for u in range(KX // 2):
                    nc.tensor.matmul(ps,
                                     wv8[:, 2 * u:2 * u + 2, mc * P:(mc + 1) * P],
                                     ctxT8[:, 2 * u:2 * u + 2, :],
                                     start=(u == 0), stop=(u == KX // 2 - 1),
                                     perf_mode=DR)
                nc.scalar.activation(vT8[:, mc, :], ps, AF.Identity,
                                     scale=1.0 / WS, bias=bkv[:, KC + mc:KC + mc + 1])

            # G [128(c'), KC, Lc] = 32 * Wq kT   (fp8; the 32 comes from wqT8
            # and is folded into the exp scale)
            G8 = const.tile([P, KC, Lc], fp8, name="G8", tag="G8")
            for mg in range(KC):
                ps = psum_s.tile([P, Lc], f32, tag="ps_s", name=f"ps_g_{mg}")
                for u in range(KC // 2):
                    nc.tensor.matmul(ps,
                                     wqT8[:, 2 * u:2 * u + 2, mg * P:(mg + 1) * P],
                                     kT8[:, 2 * u:2 * u + 2, :],
                                     start=(u == 0), stop=(u == KC // 2 - 1),
                                     perf_mode=DR)
                nc.vector.tensor_copy(out=G8[:, mg, :], in_=ps)

            # bqk_s [128(lc), KL] = SCALE * kT^T bq   (per-lc exp bias)
            bqk_s = const.tile([P, KL], f32, name="bqk_s", tag="bqk")
            for ml in range(KL):
                ps = psum_bc.tile([P, HT], f32, tag="ps_bc", name=f"ps_bq_{ml}")
                for mc in range(KC):
                    nc.tensor.matmul(ps[:, 0:1],
                                     kT8[:, mc, ml * P:(ml + 1) * P],
                                     bq8[:, mc:mc + 1],
                                     start=(mc == 0), stop=(mc == KC - 1))
                nc.scalar.activation(bqk_s[:, ml:ml + 1], ps[:, 0:1],
                                     AF.Identity, scale=SCALE)

            # vW [128(lc), KL, C] = 32*((v+bv) Wo + 1 bo^T)  (fp8; 32 from wo8,
            # cancelled by the 32 in the denominator matmul)
            vW8 = const.tile([P, KL, C], fp8, name="vW8", tag="vW8")
            for ml in range(KL):
                ps = psum_y.tile([P, C], f32, tag="ps_y", name=f"ps_vw_{ml}")
                for u in range(KC // 2):
                    nc.tensor.matmul(ps,
                                     vT8[:, 2 * u:2 * u + 2, ml * P:(ml + 1) * P],
                                     wo8[:, 2 * u:2 * u + 2, :],
                                     start=(u == 0), stop=(u == KC // 2 - 1),
                                     perf_mode=DR)
                nc.vector.tensor_add(out=vW8[:, ml, :], in0=ps, in1=bo_bc)

            # ------------- phase B: stream over hw tiles -------------
            # software pipeline: y-matmuls of tile h-1 are emitted after
            # simT of tile h, so the PE never waits on the exp evictions
            prev = None

            def emit_y(h, eT, rec_sb, xb):
                y_sb = yout.tile([P, KC, HT], bf16, tag="y", name=f"y_{h}")
                for mo in range(KC):
                    ps = psum_y.tile([P, HT], f32, tag="ps_y", name=f"ps_y_{h}_{mo}")
                    for u in range(KL // 2):
                        nc.tensor.matmul(ps,
                                         vW8[:, 2 * u:2 * u + 2, mo * P:(mo + 1) * P],
                                         eT[:, 2 * u:2 * u + 2, :],
                                         start=(u == 0), stop=(u == KL // 2 - 1),
                                         perf_mode=DR)
                    nc.vector.tensor_mul(out=y_sb[:, mo, :], in0=ps, in1=rec_sb)
                    add_eng = nc.vector if mo >= 2 else nc.gpsimd
                    add_eng.tensor_add(out=y_sb[:, mo, :], in0=y_sb[:, mo, :],
                                       in1=xb[:, mo, :])
                # one 4KB-descriptor store per tile, 8-way partition-split so
                # the final tile's store drains in ~2.5us instead of ~10us
                for q in range(8):
                    s = slice(16 * q, 16 * (q + 1))
                    nc.sync.dma_start(out=y_d[s, h], in_=y_sb[s])

            for h in range(N_HT):
                x8 = x8_tiles[h]

                # eT [lc, hw] = exp(scale*(G^T x) + scale*kT^T bq)
                eT = work.tile([P, KL, HT], fp8, tag="eT", name=f"eT_{h}")
                for ml in range(KL):
                    ps = psum_s.tile([P, HT], f32, tag="ps_s", name=f"ps_s_{h}_{ml}")
                    for u in range(KC // 2):
                        nc.tensor.matmul(ps,
                                         G8[:, 2 * u:2 * u + 2, ml * P:(ml + 1) * P],
                                         x8[:, 2 * u:2 * u + 2, :],
                                         start=(u == 0), stop=(u == KC // 2 - 1),
                                         perf_mode=DR)
                    nc.scalar.activation(eT[:, ml, :], ps, AF.Exp, scale=SCALE / WS,
                                         bias=bqk_s[:, ml:ml + 1])

                if prev is not None:
                    emit_y(*prev)

                # fused softmax denominator: one DR matmul pass over eT both
                # sums over lc and broadcasts 32*colsum to all 128 partitions
                ps_bc = psum_bc.tile([P, HT], f32, tag="ps_bc", name=f"ps_bc_{h}")
                for u in range(KL // 2):
                    nc.tensor.matmul(ps_bc, ones32, eT[:, 2 * u:2 * u + 2, :],
                                     start=(u == 0), stop=(u == KL // 2 - 1),
                                     perf_mode=DR)
                rec_sb = work.tile([P, HT], f32, tag="rec", name=f"rec_{h}")
                nc.vector.reciprocal_approx_fast(out=rec_sb, in_=ps_bc)
                prev = (h, eT, rec_sb, xb_tiles[h])
            emit_y(*prev)

    nc.compile()
    return nc


def _get_compiled():
    if "nc" not in _cache:
        _cache["nc"] = _build_nc()
    return _cache["nc"]


def _make_in_maps(x, context, Wq, bq, Wk, bk, Wv, bv, Wo, bo):
    import ml_dtypes
    fp8 = ml_dtypes.float8_e4m3
    bf16 = ml_dtypes.bfloat16

    x = np.asarray(x, dtype=np.float32)
    context = np.asarray(context, dtype=np.float32)
    Wq = np.asarray(Wq, dtype=np.float32)
    Wk = np.asarray(Wk, dtype=np.float32)
    Wv = np.asarray(Wv, dtype=np.float32)
    Wo = np.asarray(Wo, dtype=np.float32)

    def chunked(a, k):     # [k*128, F] -> [128, k, F]
        return np.ascontiguousarray(
            a.reshape(k, P, -1).transpose(1, 0, 2))

    common = {
        "wk8": chunked(WS * Wk, KX).astype(fp8),
        "wv8": chunked(WS * Wv, KX).astype(fp8),
        "wqt8": chunked(WS * Wq.T, KC).astype(fp8),
        "wo8": chunked(WS * Wo, KC).astype(fp8),
        "bq8": np.ascontiguousarray(
            np.asarray(bq, np.float32).reshape(KC, P).T).astype(fp8),
        "bkv": np.ascontiguousarray(np.stack(
            [np.asarray(bk, np.float32).reshape(KC, P),
             np.asarray(bv, np.float32).reshape(KC, P)],
        ).reshape(2 * KC, P).T),
        "bo32": np.ascontiguousarray(
            WS * np.asarray(bo, np.float32).reshape(1, C)),
    }
    in_maps = []
    for b in range(B):
        m = dict(common)
        # x2[c, hw] with c = ko*128+p  ->  [p, h, ko, j]
        x2 = x[b].reshape(KC, P, N_HT, HT).transpose(1, 2, 0, 3)
        m["x8"] = np.ascontiguousarray(x2).astype(fp8)
        m["xb"] = np.ascontiguousarray(x2).astype(bf16)
        # ctxT[cx, lc] with cx = cxo*128+p  ->  [p, cxo, lc]
        m["ctxt8"] = chunked(context[b].T, KX).astype(fp8)
        in_maps.append(m)
    return in_maps


def _run(in_maps, trace=False):
    from concourse.bass_utils import run_bass_kernel_spmd
    nc = _get_compiled()
    return run_bass_kernel_spmd(nc, in_maps, core_ids=list(range(N_CORES)),
                                trace=trace)


def _assemble(res):
    out = np.empty((B, C, HH, WW), np.float32)
    for b in range(B):
        yb = np.asarray(res.results[b]["y"], dtype=np.float32)
        # [p, h, ko, j] -> [ko, p, h, j] -> [C, HW]
        out[b] = yb.transpose(2, 0, 1, 3).reshape(C, HH, WW)
    return out


def kernel(x, context, Wq, bq, Wk, bk, Wv, bv, Wo, bo):
    in_maps = _make_in_maps(x, context, Wq, bq, Wk, bk, Wv, bv, Wo, bo)
    res = _run(in_maps, trace=False)
    return _assemble(res)


# revision 10
# speedup vs baseline: 1.7706x; 1.1411x over previous
"""Cross-attention (single-head, residual) Bass/Tile kernel for Trainium2.

Problem: y = x + (softmax((x' Wq + bq)(ctx Wk + bk)^T / sqrt(C)) (ctx Wv + bv)) Wo + bo
  x: [B=8, C=512, H=64, W=64], context: [B=8, Lc=512, CTX=768]

Sharding: pure data-parallel over batch - one batch element per NeuronCore,
no collectives.

Algebraic restructuring (saves ~1/3 of the matmul work): with
  kT = (ctx Wk + bk)^T           [C, Lc]
  G  = Wq kT                     [C', Lc]  (Wq folded into the key side)
  vW = (ctx Wv + bv) Wo + 1 bo^T [Lc, C]   (Wo and bo folded into the V side;
                                            exact because softmax rows sum to 1)
the streaming loop per hw-tile is two matmul stages plus a softmax row-sum:
  simT = G^T x                   [Lc, hw]
  eT   = exp(scale*simT + scale*kT^T bq)   (bq folded into the ACT bias)
  y    = (vW^T eT) * (1/colsum(eT)) + x

All layout work (transposes, fp8/bf16 casts, channel-chunk packing) happens
on the host; the device only runs matmuls, ACT exp/identity evictions, the
reciprocal, and the residual multiply-adds.  Weights are shipped as
fp8e4m3 scaled by 32 (raw std 0.02 is fp8-subnormal territory); the 32s
cancel against a constant-32 stationary matrix in the softmax denominator
matmul, which simultaneously computes the column sum and broadcasts it
across all 128 partitions in a single DoubleRow matmul.
"""

import numpy as np

B = 8
C = 512
CTX = 768
Lc = 512
HH = 64
WW = 64
HW = HH * WW          # 4096
N_CORES = 8
P = 128
HT = 512              # hw tile (free-dim) width
N_HT = HW // HT       # 8
KC = C // P           # 4
KX = CTX // P         # 6
KL = Lc // P          # 4
SCALE = float(C) ** -0.5
WS = 32.0             # fp8 weight pre-scale

_cache = {}


def _build_nc():
    import concourse.mybir as mybir
    import concourse.bass as bass
    import concourse.tile as tile
    from concourse import bacc

    f32 = mybir.dt.float32
    bf16 = mybir.dt.bfloat16
    fp8 = mybir.dt.float8e4
    AF = mybir.ActivationFunctionType
    DR = mybir.MatmulPerfMode.DoubleRow

    nc = bacc.Bacc("TRN2", target_bir_lowering=False, debug=False,
                   num_devices=N_CORES)

    # host-prepped layouts: [128(part), chunk, free]
    x8_d = nc.dram_tensor("x8", [P, N_HT, KC, HT], fp8, kind="ExternalInput").ap()
    xb_d = nc.dram_tensor("xb", [P, N_HT, KC, HT], bf16, kind="ExternalInput").ap()
    ctxT_d = nc.dram_tensor("ctxt8", [P, KX, Lc], fp8, kind="ExternalInput").ap()
    wk_d = nc.dram_tensor("wk8", [P, KX, C], fp8, kind="ExternalInput").ap()
    wv_d = nc.dram_tensor("wv8", [P, KX, C], fp8, kind="ExternalInput").ap()
    wqT_d = nc.dram_tensor("wqt8", [P, KC, C], fp8, kind="ExternalInput").ap()
    wo_d = nc.dram_tensor("wo8", [P, KC, C], fp8, kind="ExternalInput").ap()
    bq_d = nc.dram_tensor("bq8", [P, KC], fp8, kind="ExternalInput").ap()
    bkv_d = nc.dram_tensor("bkv", [P, 2 * KC], f32, kind="ExternalInput").ap()
    bo_d = nc.dram_tensor("bo32", [1, C], f32, kind="ExternalInput").ap()
    y_d = nc.dram_tensor("y", [P, N_HT, KC, HT], bf16, kind="ExternalOutput").ap()

    with tile.TileContext(nc) as tc:
        with (
            tc.tile_pool(name="const", bufs=1) as const,
            tc.tile_pool(name="x8in", bufs=N_HT) as x8in,
            tc.tile_pool(name="xbin", bufs=N_HT) as xbin,
            tc.tile_pool(name="work", bufs=2) as work,
            tc.tile_pool(name="yout", bufs=2) as yout,
            tc.tile_pool(name="psum_s", bufs=3, space="PSUM") as psum_s,
            tc.tile_pool(name="psum_bc", bufs=2, space="PSUM") as psum_bc,
            tc.tile_pool(name="psum_y", bufs=3, space="PSUM") as psum_y,
        ):
            # ------------- DMAs -------------
            # One dma_start lands on one hw queue (~77ns/2KB-descriptor),
            # so the two tensors gating the first matmul are 2-way
            # partition-split to halve their landing time; everything else
            # stays as few big dma_starts (descriptor generation is a
            # shared serial resource).
            ctxT8 = const.tile([P, KX, Lc], fp8, name="ctxT8", tag="ctxT8")
            wk8 = const.tile([P, KX, C], fp8, name="wk8", tag="wk8")
            for q in range(2):
                s = slice(64 * q, 64 * (q + 1))
                nc.sync.dma_start(out=ctxT8[s], in_=ctxT_d[s])
                nc.sync.dma_start(out=wk8[s], in_=wk_d[s])
            wv8 = const.tile([P, KX, C], fp8, name="wv8", tag="wv8")
            wqT8 = const.tile([P, KC, C], fp8, name="wqT8", tag="wqT8")
            wo8 = const.tile([P, KC, C], fp8, name="wo8", tag="wo8")
            nc.sync.dma_start(out=wv8, in_=wv_d)
            nc.sync.dma_start(out=wqT8, in_=wqT_d)
            nc.sync.dma_start(out=wo8, in_=wo_d)

            x8_tiles, xb_tiles = {}, {}

            def fetch_x(h):
                x8 = x8in.tile([P, KC, HT], fp8, tag="x8", name=f"x8_{h}")
                nc.sync.dma_start(out=x8, in_=x8_d[:, h])
                x8_tiles[h] = x8
                xb = xbin.tile([P, KC, HT], bf16, tag="xb", name=f"xb_{h}")
                for q in range(2):
                    s = slice(64 * q, 64 * (q + 1))
                    nc.sync.dma_start(out=xb[s], in_=xb_d[s, h])
                xb_tiles[h] = xb

            for h in range(3):
                fetch_x(h)

            # biases on the gpsimd DGE queue (tiny; keeps SP queues clean)
            bq8 = const.tile([P, KC], fp8, name="bq8", tag="bq8")
            bkv = const.tile([P, 2 * KC], f32, name="bkv", tag="bkv")
            nc.gpsimd.dma_start(out=bq8, in_=bq_d)
            nc.gpsimd.dma_start(out=bkv, in_=bkv_d)
            # 32*bo broadcast across partitions via stride-0 DMA
            bo_bc = const.tile([P, C], f32, name="bo_bc", tag="bo_bc")
            bo_src = bass.AP(tensor=bo_d.tensor, offset=bo_d.offset,
                             ap=[[0, P]] + list(bo_d.ap)[1:])
            nc.gpsimd.dma_start(out=bo_bc, in_=bo_src)

            # constant-32 stationary matrix for the fused colsum+broadcast
            ones32 = const.tile([P, 2, P], fp8, name="ones32", tag="ones32")
            nc.vector.memset(ones32, WS)

            # prewarm the ACT exp table so the one-time table load hides
            # under the initial weight DMAs
            warm = const.tile([1, 1], f32, name="warm", tag="warm")
            nc.scalar.activation(warm, ones32[0:1, 0, 0:1], AF.Exp)

            # ------------- phase A -------------
            # kT [128(c), KC, Lc] = (ctx Wk + bk)^T   (fp8, unscaled)
            kT8 = const.tile([P, KC, Lc], fp8, name="kT8", tag="kT8")
            for mc in range(KC):
                ps = psum_s.tile([P, Lc], f32, tag="ps_s", name=f"ps_k_{mc}")
                for u in range(KX // 2):
                    nc.tensor.matmul(ps,
                                     wk8[:, 2 * u:2 * u + 2, mc * P:(mc + 1) * P],
                                     ctxT8[:, 2 * u:2 * u + 2, :],
                                     start=(u == 0), stop=(u == KX // 2 - 1),
                                     perf_mode=DR)
                nc.scalar.activation(kT8[:, mc, :], ps, AF.Identity,
                                     scale=1.0 / WS, bias=bkv[:, mc:mc + 1])

            # vT [128(c), KC, Lc] = (ctx Wv + bv)^T   (fp8, unscaled)
            vT8 = const.tile([P, KC, Lc], fp8, name="vT8", tag="vT8")
            for mc in range(KC):
                ps = psum_y.tile([P, Lc], f32, tag="ps_y", name=f"ps_v_{mc}")
                for u in range(KX // 2):
                    nc.tensor.matmul(ps,
                                     wv8[:, 2 * u:2 * u + 2, mc * P:(mc + 1) * P],
                                     ctxT8[:, 2 * u:2 * u + 2, :],
                                     start=(u == 0), stop=(u == KX // 2 - 1),
                                     perf_mode=DR)
                nc.scalar.activation(vT8[:, mc, :], ps, AF.Identity,
                                     scale=1.0 / WS, bias=bkv[:, KC + mc:KC + mc + 1])

            # G [128(c'), KC, Lc] = 32 * Wq kT   (fp8; the 32 comes from wqT8
            # and is folded into the exp scale)
            G8 = const.tile([P, KC, Lc], fp8, name="G8", tag="G8")
            for mg in range(KC):
                ps = psum_s.tile([P, Lc], f32, tag="ps_s", name=f"ps_g_{mg}")
                for u in range(KC // 2):
                    nc.tensor.matmul(ps,
                                     wqT8[:, 2 * u:2 * u + 2, mg * P:(mg + 1) * P],
                                     kT8[:, 2 * u:2 * u + 2, :],
                                     start=(u == 0), stop=(u == KC // 2 - 1),
                                     perf_mode=DR)
                nc.vector.tensor_copy(out=G8[:, mg, :], in_=ps)

            # bqk_s [128(lc), KL] = SCALE * kT^T bq   (per-lc exp bias)
            bqk_s = const.tile([P, KL], f32, name="bqk_s", tag="bqk")
            for ml in range(KL):
                ps = psum_bc.tile([P, HT], f32, tag="ps_bc", name=f"ps_bq_{ml}")
                for mc in range(KC):
                    nc.tensor.matmul(ps[:, 0:1],
                                     kT8[:, mc, ml * P:(ml + 1) * P],
                                     bq8[:, mc:mc + 1],
                                     start=(mc == 0), stop=(mc == KC - 1))
                nc.scalar.activation(bqk_s[:, ml:ml + 1], ps[:, 0:1],
                                     AF.Identity, scale=SCALE)

            # vW [128(lc), KL, C] = 32*((v+bv) Wo + 1 bo^T)  (fp8; 32 from wo8,
            # cancelled by the 32 in the denominator matmul)
            vW8 = const.tile([P, KL, C], fp8, name="vW8", tag="vW8")
            for ml in range(KL):
                ps = psum_y.tile([P, C], f32, tag="ps_y", name=f"ps_vw_{ml}")
                for u in range(KC // 2):
                    nc.tensor.matmul(ps,
                                     vT8[:, 2 * u:2 * u + 2, ml * P:(ml + 1) * P],
                                     wo8[:, 2 * u:2 * u + 2, :],
                                     start=(u == 0), stop=(u == KC // 2 - 1),
                                     perf_mode=DR)
                nc.vector.tensor_add(out=vW8[:, ml, :], in0=ps, in1=bo_bc)

            # ------------- phase B: stream over hw tiles -------------
            # software pipeline: y-matmuls of tile h-1 are emitted after
            # simT of tile h, so the PE never waits on the exp evictions
            prev = None

            def emit_y(h, eT, rec_sb, xb):
                y_sb = yout.tile([P, KC, HT], bf16, tag="y", name=f"y_{h}")
                for mo in range(KC):
                    ps = psum_y.tile([P, HT], f32, tag="ps_y", name=f"ps_y_{h}_{mo}")
                    for u in range(KL // 2):
                        nc.tensor.matmul(ps,
                                         vW8[:, 2 * u:2 * u + 2, mo * P:(mo + 1) * P],
                                         eT[:, 2 * u:2 * u + 2, :],
                                         start=(u == 0), stop=(u == KL // 2 - 1),
                                         perf_mode=DR)
                    nc.vector.tensor_mul(out=y_sb[:, mo, :], in0=ps, in1=rec_sb)
                    add_eng = nc.vector if mo >= 2 else nc.gpsimd
                    add_eng.tensor_add(out=y_sb[:, mo, :], in0=y_sb[:, mo, :],
                                       in1=xb[:, mo, :])
                # per-tile store with 4KB-contiguous runs; the final tile is
                # 8-way partition-split so the drain tail is ~2.5us not ~10us
                nsplit = 8 if h == N_HT - 1 else 2
                w = P // nsplit
                for q in range(nsplit):
                    s = slice(w * q, w * (q + 1))
                    nc.sync.dma_start(out=y_d[s, h], in_=y_sb[s])

            for h in range(N_HT):
                if h + 3 < N_HT:
                    fetch_x(h + 3)
                x8 = x8_tiles[h]

                # eT [lc, hw] = exp(scale*(G^T x) + scale*kT^T bq)
                eT = work.tile([P, KL, HT], fp8, tag="eT", name=f"eT_{h}")
                for ml in range(KL):
                    ps = psum_s.tile([P, HT], f32, tag="ps_s", name=f"ps_s_{h}_{ml}")
                    for u in range(KC // 2):
                        nc.tensor.matmul(ps,
                                         G8[:, 2 * u:2 * u + 2, ml * P:(ml + 1) * P],
                                         x8[:, 2 * u:2 * u + 2, :],
                                         start=(u == 0), stop=(u == KC // 2 - 1),
                                         perf_mode=DR)
                    nc.scalar.activation(eT[:, ml, :], ps, AF.Exp, scale=SCALE / WS,
                                         bias=bqk_s[:, ml:ml + 1])

                if prev is not None:
                    emit_y(*prev)

                # fused softmax denominator: one DR matmul pass over eT both
                # sums over lc and broadcasts 32*colsum to all 128 partitions
                ps_bc = psum_bc.tile([P, HT], f32, tag="ps_bc", name=f"ps_bc_{h}")
                for u in range(KL // 2):
                    nc.tensor.matmul(ps_bc, ones32, eT[:, 2 * u:2 * u + 2, :],
                                     start=(u == 0), stop=(u == KL // 2 - 1),
                                     perf_mode=DR)
                rec_sb = work.tile([P, HT], f32, tag="rec", name=f"rec_{h}")
                nc.vector.reciprocal_approx_fast(out=rec_sb, in_=ps_bc)
                prev = (h, eT, rec_sb, xb_tiles[h])
            emit_y(*prev)

    nc.compile()
    return nc


def _get_compiled():
    if "nc" not in _cache:
        _cache["nc"] = _build_nc()
    return _cache["nc"]


def _make_in_maps(x, context, Wq, bq, Wk, bk, Wv, bv, Wo, bo):
    import ml_dtypes
    fp8 = ml_dtypes.float8_e4m3
    bf16 = ml_dtypes.bfloat16

    x = np.asarray(x, dtype=np.float32)
    context = np.asarray(context, dtype=np.float32)
    Wq = np.asarray(Wq, dtype=np.float32)
    Wk = np.asarray(Wk, dtype=np.float32)
    Wv = np.asarray(Wv, dtype=np.float32)
    Wo = np.asarray(Wo, dtype=np.float32)

    def chunked(a, k):     # [k*128, F] -> [128, k, F]
        return np.ascontiguousarray(
            a.reshape(k, P, -1).transpose(1, 0, 2))

    common = {
        "wk8": chunked(WS * Wk, KX).astype(fp8),
        "wv8": chunked(WS * Wv, KX).astype(fp8),
        "wqt8": chunked(WS * Wq.T, KC).astype(fp8),
        "wo8": chunked(WS * Wo, KC).astype(fp8),
        "bq8": np.ascontiguousarray(
            np.asarray(bq, np.float32).reshape(KC, P).T).astype(fp8),
        "bkv": np.ascontiguousarray(np.stack(
            [np.asarray(bk, np.float32).reshape(KC, P),
             np.asarray(bv, np.float32).reshape(KC, P)],
        ).reshape(2 * KC, P).T),
        "bo32": np.ascontiguousarray(
            WS * np.asarray(bo, np.float32).reshape(1, C)),
    }
    in_maps = []
    for b in range(B):
        m = dict(common)
        # x2[c, hw] with c = ko*128+p  ->  [p, h, ko, j]
        x2 = x[b].reshape(KC, P, N_HT, HT).transpose(1, 2, 0, 3)
        m["x8"] = np.ascontiguousarray(x2).astype(fp8)
        m["xb"] = np.ascontiguousarray(x2).astype(bf16)
        # ctxT[cx, lc] with cx = cxo*128+p  ->  [p, cxo, lc]
        m["ctxt8"] = chunked(context[b].T, KX).astype(fp8)
        in_maps.append(m)
    return in_maps


def _run(in_maps, trace=False):
    from concourse.bass_utils import run_bass_kernel_spmd
    nc = _get_compiled()
    return run_bass_kernel_spmd(nc, in_maps, core_ids=list(range(N_CORES)),
                                trace=trace)


def _assemble(res):
    out = np.empty((B, C, HH, WW), np.float32)
    for b in range(B):
        yb = np.asarray(res.results[b]["y"], dtype=np.float32)
        # [p, h, ko, j] -> [ko, p, h, j] -> [C, HW]
        out[b] = yb.transpose(2, 0, 1, 3).reshape(C, HH, WW)
    return out


def kernel(x, context, Wq, bq, Wk, bk, Wv, bv, Wo, bo):
    in_maps = _make_in_maps(x, context, Wq, bq, Wk, bk, Wv, bv, Wo, bo)
    res = _run(in_maps, trace=False)
    return _assemble(res)


# revision 15
# speedup vs baseline: 1.8090x; 1.0217x over previous
"""Cross-attention (single-head, residual) Bass/Tile kernel for Trainium2.

Problem: y = x + (softmax((x' Wq + bq)(ctx Wk + bk)^T / sqrt(C)) (ctx Wv + bv)) Wo + bo
  x: [B=8, C=512, H=64, W=64], context: [B=8, Lc=512, CTX=768]

Sharding: pure data-parallel over batch - one batch element per NeuronCore,
no collectives.

Algebraic restructuring (saves ~1/3 of the matmul work): with
  kT = (ctx Wk + bk)^T           [C, Lc]
  G  = Wq kT                     [C', Lc]  (Wq folded into the key side)
  vW = (ctx Wv + bv) Wo + 1 bo^T [Lc, C]   (Wo and bo folded into the V side;
                                            exact because softmax rows sum to 1)
the streaming loop per hw-tile is two matmul stages plus a softmax row-sum:
  simT = G^T x                   [Lc, hw]
  eT   = exp(scale*simT + scale*kT^T bq)   (bq folded into the ACT bias)
  y    = (vW^T eT) * (1/colsum(eT)) + x

All layout work (transposes, fp8/bf16 casts, channel-chunk packing) happens
on the host; the device only runs matmuls, ACT exp/identity evictions, the
reciprocal, and the residual multiply-adds.  Weights are shipped as
fp8e4m3 scaled by 32 (raw std 0.02 is fp8-subnormal territory); the 32s
cancel against a constant-32 stationary matrix in the softmax denominator
matmul, which simultaneously computes the column sum and broadcasts it
across all 128 partitions in a single DoubleRow matmul.
"""

import numpy as np

B = 8
C = 512
CTX = 768
Lc = 512
HH = 64
WW = 64
HW = HH * WW          # 4096
N_CORES = 8
P = 128
HT = 512              # hw tile (free-dim) width
N_HT = HW // HT       # 8
KC = C // P           # 4
KX = CTX // P         # 6
KL = Lc // P          # 4
SCALE = float(C) ** -0.5
WS = 32.0             # fp8 weight pre-scale

_cache = {}


def _build_nc():
    import concourse.mybir as mybir
    import concourse.bass as bass
    import concourse.tile as tile
    from concourse import bacc

    f32 = mybir.dt.float32
    bf16 = mybir.dt.bfloat16
    fp8 = mybir.dt.float8e4
    AF = mybir.ActivationFunctionType
    DR = mybir.MatmulPerfMode.DoubleRow

    nc = bacc.Bacc("TRN2", target_bir_lowering=False, debug=False,
                   num_devices=N_CORES)

    # host-prepped layouts: [128(part), chunk, free]
    x8_d = nc.dram_tensor("x8", [P, N_HT, KC, HT], fp8, kind="ExternalInput").ap()
    xb_d = nc.dram_tensor("xb", [P, N_HT, KC, HT], bf16, kind="ExternalInput").ap()
    ctxT_d = nc.dram_tensor("ctxt8", [P, KX, Lc], fp8, kind="ExternalInput").ap()
    wk_d = nc.dram_tensor("wk8", [P, KX, C], fp8, kind="ExternalInput").ap()
    wv_d = nc.dram_tensor("wv8", [P, KX, C], fp8, kind="ExternalInput").ap()
    wqT_d = nc.dram_tensor("wqt8", [P, KC, C], fp8, kind="ExternalInput").ap()
    wo_d = nc.dram_tensor("wo8", [P, KC, C], fp8, kind="ExternalInput").ap()
    bq_d = nc.dram_tensor("bq8", [P, KC], fp8, kind="ExternalInput").ap()
    bkv_d = nc.dram_tensor("bkv", [P, 2 * KC], f32, kind="ExternalInput").ap()
    bo_d = nc.dram_tensor("bo32", [1, C], f32, kind="ExternalInput").ap()
    y_d = nc.dram_tensor("y", [P, N_HT, KC, HT], bf16, kind="ExternalOutput").ap()

    with tile.TileContext(nc) as tc:
        with (
            tc.tile_pool(name="const", bufs=1) as const,
            tc.tile_pool(name="x8in", bufs=N_HT) as x8in,
            tc.tile_pool(name="xbin", bufs=N_HT) as xbin,
            tc.tile_pool(name="work", bufs=2) as work,
            tc.tile_pool(name="yout", bufs=2) as yout,
            tc.tile_pool(name="psum_s", bufs=4, space="PSUM") as psum_s,
            tc.tile_pool(name="psum_bc", bufs=1, space="PSUM") as psum_bc,
            tc.tile_pool(name="psum_y", bufs=3, space="PSUM") as psum_y,
        ):
            # ------------- DMAs -------------
            # ctx/wk arrive as cx-chunk-pair pieces matching the
            # chunk-major kT loop below: the first kT matmuls only wait for
            # the first pair (0.26MB), not the whole 0.77MB.
            ctxT8 = const.tile([P, KX, Lc], fp8, name="ctxT8", tag="ctxT8")
            wk8 = const.tile([P, KX, C], fp8, name="wk8", tag="wk8")
            for u in range(KX // 2):
                cp = slice(2 * u, 2 * u + 2)
                nc.sync.dma_start(out=wk8[:, cp, :], in_=wk_d[:, cp, :])
                nc.sync.dma_start(out=ctxT8[:, cp, :], in_=ctxT_d[:, cp, :])
            wv8 = const.tile([P, KX, C], fp8, name="wv8", tag="wv8")
            wqT8 = const.tile([P, KC, C], fp8, name="wqT8", tag="wqT8")
            wo8 = const.tile([P, KC, C], fp8, name="wo8", tag="wo8")
            nc.sync.dma_start(out=wv8, in_=wv_d)
            nc.sync.dma_start(out=wqT8, in_=wqT_d)
            nc.sync.dma_start(out=wo8, in_=wo_d)

            x8_tiles, xb_tiles = {}, {}

            def fetch_x(h):
                x8 = x8in.tile([P, KC, HT], fp8, tag="x8", name=f"x8_{h}")
                nc.sync.dma_start(out=x8, in_=x8_d[:, h])
                x8_tiles[h] = x8
                xb = xbin.tile([P, KC, HT], bf16, tag="xb", name=f"xb_{h}")
                for q in range(2):
                    s = slice(64 * q, 64 * (q + 1))
                    nc.sync.dma_start(out=xb[s], in_=xb_d[s, h])
                xb_tiles[h] = xb

            for h in range(4):
                fetch_x(h)

            # biases on the gpsimd DGE queue (tiny; keeps SP queues clean)
            bq8 = const.tile([P, KC], fp8, name="bq8", tag="bq8")
            bkv = const.tile([P, 2 * KC], f32, name="bkv", tag="bkv")
            nc.gpsimd.dma_start(out=bq8, in_=bq_d)
            nc.gpsimd.dma_start(out=bkv, in_=bkv_d)
            # 32*bo broadcast across partitions via stride-0 DMA
            bo_bc = const.tile([P, C], f32, name="bo_bc", tag="bo_bc")
            bo_src = bass.AP(tensor=bo_d.tensor, offset=bo_d.offset,
                             ap=[[0, P]] + list(bo_d.ap)[1:])
            nc.gpsimd.dma_start(out=bo_bc, in_=bo_src)

            # constant-32 stationary matrix for the fused colsum+broadcast
            ones32 = const.tile([P, 2, P], fp8, name="ones32", tag="ones32")
            nc.vector.memset(ones32, WS)

            # prewarm the ACT exp table so the one-time table load hides
            # under the initial weight DMAs
            warm = const.tile([1, 1], f32, name="warm", tag="warm")
            nc.scalar.activation(warm, ones32[0:1, 0, 0:1], AF.Exp)

            # ------------- phase A -------------
            # kT [128(c), KC, Lc] = (ctx Wk + bk)^T   (fp8, unscaled)
            # chunk-major: the u=0 sweep over all 4 mc only needs the first
            # cx-pair of wk8/ctxT8, so the PE starts ~7us earlier
            kT8 = const.tile([P, KC, Lc], fp8, name="kT8", tag="kT8")
            kps = [psum_s.tile([P, Lc], f32, tag="ps_s", name=f"ps_k_{mc}")
                   for mc in range(KC)]
            for u in range(KX // 2):
                for mc in range(KC):
                    nc.tensor.matmul(kps[mc],
                                     wk8[:, 2 * u:2 * u + 2, mc * P:(mc + 1) * P],
                                     ctxT8[:, 2 * u:2 * u + 2, :],
                                     start=(u == 0), stop=(u == KX // 2 - 1),
                                     perf_mode=DR)
            for mc in range(KC):
                nc.scalar.activation(kT8[:, mc, :], kps[mc], AF.Identity,
                                     scale=1.0 / WS, bias=bkv[:, mc:mc + 1])

            # vT [128(c), KC, Lc] = (ctx Wv + bv)^T   (fp8, unscaled)
            vT8 = const.tile([P, KC, Lc], fp8, name="vT8", tag="vT8")
            for mc in range(KC):
                ps = psum_y.tile([P, Lc], f32, tag="ps_y", name=f"ps_v_{mc}")
                for u in range(KX // 2):
                    nc.tensor.matmul(ps,
                                     wv8[:, 2 * u:2 * u + 2, mc * P:(mc + 1) * P],
                                     ctxT8[:, 2 * u:2 * u + 2, :],
                                     start=(u == 0), stop=(u == KX // 2 - 1),
                                     perf_mode=DR)
                nc.scalar.activation(vT8[:, mc, :], ps, AF.Identity,
                                     scale=1.0 / WS, bias=bkv[:, KC + mc:KC + mc + 1])

            # G [128(c'), KC, Lc] = 32 * Wq kT   (fp8; the 32 comes from wqT8
            # and is folded into the exp scale)
            G8 = const.tile([P, KC, Lc], fp8, name="G8", tag="G8")
            for mg in range(KC):
                ps = psum_s.tile([P, Lc], f32, tag="ps_s", name=f"ps_g_{mg}")
                for u in range(KC // 2):
                    nc.tensor.matmul(ps,
                                     wqT8[:, 2 * u:2 * u + 2, mg * P:(mg + 1) * P],
                                     kT8[:, 2 * u:2 * u + 2, :],
                                     start=(u == 0), stop=(u == KC // 2 - 1),
                                     perf_mode=DR)
                nc.vector.tensor_copy(out=G8[:, mg, :], in_=ps)

            # bqk_s [128(lc), KL] = SCALE * kT^T bq   (per-lc exp bias)
            bqk_s = const.tile([P, KL], f32, name="bqk_s", tag="bqk")
            for ml in range(KL):
                ps = psum_bc.tile([P, HT], f32, tag="ps_bc", name=f"ps_bq_{ml}")
                for mc in range(KC):
                    nc.tensor.matmul(ps[:, 0:1],
                                     kT8[:, mc, ml * P:(ml + 1) * P],
                                     bq8[:, mc:mc + 1],
                                     start=(mc == 0), stop=(mc == KC - 1))
                nc.scalar.activation(bqk_s[:, ml:ml + 1], ps[:, 0:1],
                                     AF.Identity, scale=SCALE)

            # vW [128(lc), KL, C] = 32*((v+bv) Wo + 1 bo^T)  (fp8; 32 from wo8,
            # cancelled by the 32 in the denominator matmul)
            vW8 = const.tile([P, KL, C], fp8, name="vW8", tag="vW8")
            for ml in range(KL):
                ps = psum_y.tile([P, C], f32, tag="ps_y", name=f"ps_vw_{ml}")
                for u in range(KC // 2):
                    nc.tensor.matmul(ps,
                                     vT8[:, 2 * u:2 * u + 2, ml * P:(ml + 1) * P],
                                     wo8[:, 2 * u:2 * u + 2, :],
                                     start=(u == 0), stop=(u == KC // 2 - 1),
                                     perf_mode=DR)
                nc.vector.tensor_add(out=vW8[:, ml, :], in0=ps, in1=bo_bc)

            # ------------- phase B: stream over hw tiles -------------
            # software pipeline: y-matmuls of tile h-1 are emitted after
            # simT of tile h, so the PE never waits on the exp evictions
            prev = None

            def emit_y(h, eT, rec_sb, xb):
                y_sb = yout.tile([P, KC, HT], bf16, tag="y", name=f"y_{h}")
                for mo in range(KC):
                    ps = psum_y.tile([P, HT], f32, tag="ps_y", name=f"ps_y_{h}_{mo}")
                    for u in range(KL // 2):
                        nc.tensor.matmul(ps,
                                         vW8[:, 2 * u:2 * u + 2, mo * P:(mo + 1) * P],
                                         eT[:, 2 * u:2 * u + 2, :],
                                         start=(u == 0), stop=(u == KL // 2 - 1),
                                         perf_mode=DR)
                    nc.vector.tensor_mul(out=y_sb[:, mo, :], in0=ps, in1=rec_sb)
                    add_eng = nc.vector if mo >= 2 else nc.gpsimd
                    add_eng.tensor_add(out=y_sb[:, mo, :], in0=y_sb[:, mo, :],
                                       in1=xb[:, mo, :])
                # per-tile store with 4KB-contiguous runs; the final tile is
                # 8-way partition-split so the drain tail is ~2.5us not ~10us
                nsplit = 8 if h == N_HT - 1 else 2
                w = P // nsplit
                for q in range(nsplit):
                    s = slice(w * q, w * (q + 1))
                    nc.sync.dma_start(out=y_d[s, h], in_=y_sb[s])

            for h in range(N_HT):
                if h + 4 < N_HT:
                    fetch_x(h + 4)
                x8 = x8_tiles[h]

                # eT [lc, hw] = exp(scale*(G^T x) + scale*kT^T bq)
                eT = work.tile([P, KL, HT], fp8, tag="eT", name=f"eT_{h}")
                for ml in range(KL):
                    ps = psum_s.tile([P, HT], f32, tag="ps_s", name=f"ps_s_{h}_{ml}")
                    for u in range(KC // 2):
                        nc.tensor.matmul(ps,
                                         G8[:, 2 * u:2 * u + 2, ml * P:(ml + 1) * P],
                                         x8[:, 2 * u:2 * u + 2, :],
                                         start=(u == 0), stop=(u == KC // 2 - 1),
                                         perf_mode=DR)
                    nc.scalar.activation(eT[:, ml, :], ps, AF.Exp, scale=SCALE / WS,
                                         bias=bqk_s[:, ml:ml + 1])

                if prev is not None:
                    emit_y(*prev)

                # fused softmax denominator: one DR matmul pass over eT both
                # sums over lc and broadcasts 32*colsum to all 128 partitions
                ps_bc = psum_bc.tile([P, HT], f32, tag="ps_bc", name=f"ps_bc_{h}")
                for u in range(KL // 2):
                    nc.tensor.matmul(ps_bc, ones32, eT[:, 2 * u:2 * u + 2, :],
                                     start=(u == 0), stop=(u == KL // 2 - 1),
                                     perf_mode=DR)
                rec_sb = work.tile([P, HT], f32, tag="rec", name=f"rec_{h}")
                nc.vector.reciprocal_approx_fast(out=rec_sb, in_=ps_bc)
                prev = (h, eT, rec_sb, xb_tiles[h])
            emit_y(*prev)

    nc.compile()
    return nc


def _get_compiled():
    if "nc" not in _cache:
        _cache["nc"] = _build_nc()
    return _cache["nc"]


def _make_in_maps(x, context, Wq, bq, Wk, bk, Wv, bv, Wo, bo):
    import ml_dtypes
    fp8 = ml_dtypes.float8_e4m3
    bf16 = ml_dtypes.bfloat16

    x = np.asarray(x, dtype=np.float32)
    context = np.asarray(context, dtype=np.float32)
    Wq = np.asarray(Wq, dtype=np.float32)
    Wk = np.asarray(Wk, dtype=np.float32)
    Wv = np.asarray(Wv, dtype=np.float32)
    Wo = np.asarray(Wo, dtype=np.float32)

    def chunked(a, k):     # [k*128, F] -> [128, k, F]
        return np.ascontiguousarray(
            a.reshape(k, P, -1).transpose(1, 0, 2))

    common = {
        "wk8": chunked(WS * Wk, KX).astype(fp8),
        "wv8": chunked(WS * Wv, KX).astype(fp8),
        "wqt8": chunked(WS * Wq.T, KC).astype(fp8),
        "wo8": chunked(WS * Wo, KC).astype(fp8),
        "bq8": np.ascontiguousarray(
            np.asarray(bq, np.float32).reshape(KC, P).T).astype(fp8),
        "bkv": np.ascontiguousarray(np.stack(
            [np.asarray(bk, np.float32).reshape(KC, P),
             np.asarray(bv, np.float32).reshape(KC, P)],
        ).reshape(2 * KC, P).T),
        "bo32": np.ascontiguousarray(
            WS * np.asarray(bo, np.float32).reshape(1, C)),
    }
    in_maps = []
    for b in range(B):
        m = dict(common)
        # x2[c, hw] with c = ko*128+p  ->  [p, h, ko, j]
        x2 = x[b].reshape(KC, P, N_HT, HT).transpose(1, 2, 0, 3)
        m["x8"] = np.ascontiguousarray(x2).astype(fp8)
        m["xb"] = np.ascontiguousarray(x2).astype(bf16)
        # ctxT[cx, lc] with cx = cxo*128+p  ->  [p, cxo, lc]
        m["ctxt8"] = chunked(context[b].T, KX).astype(fp8)
        in_maps.append(m)
    return in_maps


def _run(in_maps, trace=False):
    from concourse.bass_utils import run_bass_kernel_spmd
    nc = _get_compiled()
    return run_bass_kernel_spmd(nc, in_maps, core_ids=list(range(N_CORES)),
                                trace=trace)


def _assemble(res):
    out = np.empty((B, C, HH, WW), np.float32)
    for b in range(B):
        yb = np.asarray(res.results[b]["y"], dtype=np.float32)
        # [p, h, ko, j] -> [ko, p, h, j] -> [C, HW]
        out[b] = yb.transpose(2, 0, 1, 3).reshape(C, HH, WW)
    return out


def kernel(x, context, Wq, bq, Wk, bk, Wv, bv, Wo, bo):
    in_maps = _make_in_maps(x, context, Wq, bq, Wk, bk, Wv, bv, Wo, bo)
    res = _run(in_maps, trace=False)
    return _assemble(res)
